# revision 25
# baseline (speedup 1.0000x reference)
"""GNN neighbor-max kernel — bf16 ap_gather + pair-max-tree design.

Per core: 2 samples, batch-parallel across the 8 NeuronCores. Per sample:
  bf16 table xe[16g+q, m, j] = x[8q+j, (m + 512g) % N]  (one copy per GPSIMD
  group, rolled by the group's node base so each group's own nodes sit at
  uniform offsets; 64KB/partition, double-buffered across samples).
  Group g owns nodes [g*512, (g+1)*512); its per-chunk index list holds the
  16 neighbors of 32 nodes, pre-shifted by -512g mod N: one ap_gather of 512
  idx -> gt [128, 512*8] bf16.
  DVE reduces k=16 via a contiguous pair-max tree (16->8->4->2->1, 2-byte
  packed innermost so the DVE 2x mode applies), then a final max against the
  table's own x slice (self node, uniform offset thanks to the roll) writes
  transposed into oblk [128, (j, n)].
  Per-sample oblk buffers; scalar engine drains uneven waves (5/8 then 3/8 of
  nodes) to out[C, N] bf16 so the post-last-gather tail is short.
  A tiny warmup ap_gather on zeroed scratch runs before the table wait to
  pre-pay MODIFY_POOL_CONFIG + the ~6us ucode IRAM load.

  Perf notes (HW-measured): the kernel is bound by ap_gather at ~39 cyc/idx
  (~63 GB/s/core of gathered bytes); dma_gather alternatives measure 26-31
  GB/s (SWDGE descriptor-generation bound) and multi-queue SWDGE corrupts
  data, so this architecture is the practical optimum with stock ucode.
"""

import numpy as np
import ml_dtypes

import concourse.bacc as bacc
import concourse.bass as bass
import concourse.mybir as mybir
from concourse.bass_utils import run_bass_kernel_spmd

B, C, N, K = 16, 128, 4096, 16
N_CORES = 8
S = B // N_CORES
D = 8                      # channels per partition block
NG = 8                     # gpsimd groups
NPG = N // NG              # 512 nodes per group
CHUNKS = 8
NPC = NPG // CHUNKS        # 32 nodes per group per chunk
NI = NPC * K               # 512 idx per gather
NCOLS = NPG * K // 16      # 512 idx columns per sample per partition

_NC_CACHE = {}


def _build_program():
    nc = bacc.Bacc(None, target_bir_lowering=False)

    bf16 = mybir.dt.bfloat16
    xe_d = nc.dram_tensor("xe", [S, C, N * D], bf16, kind="ExternalInput")
    idx_d = nc.dram_tensor("idx", [S, C, NCOLS], mybir.dt.int16,
                           kind="ExternalInput")
    # compact dedup'd table for sample-0 chunk-0 (starts the gather stream
    # ~16us before the full tbl0 lands); loaded into gt2 (dead until chunk 2)
    xcomp_d = nc.dram_tensor("xcomp", [C, NI * D], bf16, kind="ExternalInput")
    out_d = nc.dram_tensor("out", [S, C, N], bf16, kind="ExternalOutput")

    with (
        nc.Block() as block,
        nc.semaphore("isem") as isem,   # idx DMAs
        nc.semaphore("csem") as csem,   # compact chunk-0 table DMA
        nc.semaphore("tsem") as tsem,   # table DMAs
        nc.semaphore("gsem") as gsem,   # gather chunks done
        nc.semaphore("bsem") as bsem,   # tree level A done (gt free)
        nc.semaphore("msem") as msem,   # per-half-sample maxes done
        nc.semaphore("osem") as osem,   # out DMAs done
        nc.sbuf_tensor("tbl0", [C, N * D], bf16) as tbl0,      # 64KB/p
        nc.sbuf_tensor("tbl1", [C, N * D], bf16) as tbl1,      # 64KB/p
        nc.sbuf_tensor("gt0", [C, NI * D], bf16) as gt0,       # 16KB/p
        nc.sbuf_tensor("gt1", [C, NI * D], bf16) as gt1,
        nc.sbuf_tensor("gt2", [C, NI * D], bf16) as gt2,
        nc.sbuf_tensor("tA", [C, NPC * 8 * D], bf16) as tA,    # 8KB/p
        nc.sbuf_tensor("tB", [C, NPC * 4 * D], bf16) as tB,
        nc.sbuf_tensor("ob0", [C, D * NPG], bf16) as ob0,      # 8KB/p
        nc.sbuf_tensor("ob1", [C, D * NPG], bf16) as ob1,
        nc.sbuf_tensor("idxt", [C, S * NCOLS], mybir.dt.int16) as idxt,
        nc.sbuf_tensor("wg", [C, 32], mybir.dt.int16) as wg,
        nc.sbuf_tensor("wo", [C, 32], mybir.dt.int16) as wo,
    ):
        tbls = [tbl0, tbl1]
        gts = [gt0, gt1, gt2]
        obs = [ob0, ob1]

        def drain_wave(eng, s, h, r0, r1, groups):
            for gg in groups:
                src = obs[s][gg * 16:(gg + 1) * 16].rearrange(
                    "p (j n) -> p j n", j=D)[:, :, r0:r1]
                dst = bass.AP(
                    out_d,
                    s * C * N + gg * NPG + r0,
                    [[D * N, 16], [N, D], [1, r1 - r0]],
                )
                eng.dma_start(out=dst, in_=src).then_inc(osem, 16)

        waves = [(0, 5 * NPC), (5 * NPC, NPG)]

        @block.sync
        def _(sy: bass.BassEngine):
            for s in range(S):
                sy.dma_start(out=idxt[:, s * NCOLS:(s + 1) * NCOLS],
                             in_=idx_d[s]).then_inc(isem, 16)
            sy.dma_start(out=gts[2][:], in_=xcomp_d[:]).then_inc(csem, 16)
            for s in range(S):
                sy.dma_start(out=tbls[s][:], in_=xe_d[s]).then_inc(tsem, 16)
            # sync engine drains groups 4-7 of each wave in parallel with
            # the scalar engine's groups 0-3 (halves the issue serialization)
            for s in range(S):
                for h, (r0, r1) in enumerate(waves):
                    sy.wait_ge(msem, 2 * s + h + 1)
                    drain_wave(sy, s, h, r0, r1, range(NG // 2, NG))
            sy.wait_ge(osem, 16 * NG * 2 * S)

        @block.gpsimd
        def _(g: bass.BassGpSimd):
            # warmup: pre-pay MODIFY_POOL_CONFIG + ~6us IRAM load while the
            # tables are still streaming in (gathers 16 zero-idx from zeros)
            g.memset(wg[:], 0)
            g.ap_gather(
                out_ap=wo[:].rearrange("p (n j) -> p n j", j=2),
                in_ap=wg[:].rearrange("p (n j) -> p n j", j=2),
                idxs_ap=wg[:, 0:1],
                channels=C, num_elems=16, d=2, num_idxs=16,
            )
            g.wait_ge(isem, 16 * S)
            # chunk (0,0) gathers from the compact table in gt2 — starts as
            # soon as the 2MB compact DMA lands, well before tbl0
            g.wait_ge(csem, 16)
            g.ap_gather(
                out_ap=gts[0][:],
                in_ap=gts[2][:],
                idxs_ap=idxt[:, 0:NI // 16],
                channels=C, num_elems=NI, d=D, num_idxs=NI,
            ).then_inc(gsem, 1)
            for s in range(S):
                g.wait_ge(tsem, 16 * (s + 1))
                for c in range(CHUNKS):
                    ci = s * CHUNKS + c
                    if ci == 0:
                        continue
                    if ci >= 3:
                        g.wait_ge(bsem, ci - 2)
                    col0 = s * NCOLS + c * (NI // 16)
                    g.ap_gather(
                        out_ap=gts[ci % 3][:],
                        in_ap=tbls[s][:],
                        idxs_ap=idxt[:, col0:col0 + NI // 16],
                        channels=C, num_elems=N, d=D, num_idxs=NI,
                    ).then_inc(gsem, 1)

        @block.vector
        def _(v: bass.BassVectorEngine):
            def tree(s, c, buf, n0, npc, gtarget, inc_msem):
                """Pair-max tree over nodes [n0, n0+npc) of chunk (s, c)."""
                v.wait_ge(gsem, gtarget)
                gv = gts[buf][:, n0 * K * D:(n0 + npc) * K * D].rearrange(
                    "p (n k j) -> p n k j", k=K, j=D)
                av = tA[:, :npc * 8 * D].rearrange(
                    "p (n t j) -> p n t j", t=8, j=D)
                bv = tB[:, :npc * 4 * D].rearrange(
                    "p (n t j) -> p n t j", t=4, j=D)
                # C/D reuse tA: op B consumed all of tA before C writes it
                cv = tA[:, :npc * 2 * D].rearrange(
                    "p (n t j) -> p n t j", t=2, j=D)
                dv = tA[:, npc * 2 * D:npc * 3 * D].rearrange(
                    "p (n j) -> p n j", j=D)
                v.tensor_max(out=av, in0=gv[:, :, 0:K:2, :],
                             in1=gv[:, :, 1:K:2, :]).then_inc(bsem, 1)
                v.tensor_max(out=bv, in0=av[:, :, 0:8:2, :],
                             in1=av[:, :, 1:8:2, :])
                v.tensor_max(out=cv, in0=bv[:, :, 0:4:2, :],
                             in1=bv[:, :, 1:4:2, :])
                v.tensor_max(out=dv, in0=cv[:, :, 0, :],
                             in1=cv[:, :, 1, :])
                lo = c * NPC + n0
                sv = tbls[s][:].rearrange("p (n j) -> p n j", j=D)[
                    :, lo:lo + npc, :]
                ov = obs[s][:].rearrange("p (j n) -> p j n", j=D)
                ov = ov.transpose([0, 2, 1])[:, lo:lo + npc, :]
                e = v.tensor_max(out=ov, in0=dv, in1=sv)
                if inc_msem:
                    e.then_inc(msem, 1)

            for s in range(S):
                for c in range(CHUNKS):
                    ci = s * CHUNKS + c
                    if ci == 0:
                        # chunk 0 gathered from the compact table; its
                        # self-max still reads tbl0 — ensure it landed
                        v.wait_ge(tsem, 16)
                    tree(s, c, ci % 3, 0, NPC, ci + 1,
                         c == 4 or c == CHUNKS - 1)

        @block.scalar
        def _(sc: bass.BassEngine):
            # uneven waves: first drain covers chunks 0-4 (5*NPC nodes) so the
            # post-last-gather tail only moves the remaining 3*NPC nodes
            for s in range(S):
                for h, (r0, r1) in enumerate(waves):
                    sc.wait_ge(msem, 2 * s + h + 1)
                    drain_wave(sc, s, h, r0, r1, range(NG // 2))
            sc.wait_ge(osem, 16 * NG * 2 * S)

    nc.compile()
    return nc


def _prep_sample(x_s: np.ndarray, nidx_s: np.ndarray, compact_chunk0=False):
    """x_s [C, N] f32, nidx_s [N, K] int -> (xe [C, N*D] bf16, idx [C, NCOLS]
    i16, xcomp [C, NI*D] bf16 | None).

    With compact_chunk0, chunk 0's idx are remapped into a dedup'd compact
    column table (xcomp) so the first gather can run before the full table
    DMA completes."""
    xq = x_s.reshape(16, D, N).transpose(0, 2, 1)          # [q, n, j]
    xe4 = np.empty((NG, 16, N, D), dtype=np.float32)
    for g in range(NG):
        xe4[g] = np.roll(xq, -NPG * g, axis=1)             # group-rolled copy
    xe4 = xe4.astype(ml_dtypes.bfloat16)
    xe = np.ascontiguousarray(xe4.reshape(C, N * D))
    nidx = np.asarray(nidx_s, dtype=np.int64)              # [N, K]
    blocks = []
    xcomp = None
    if compact_chunk0:
        xcomp = np.zeros((NG, 16, NI, D), dtype=ml_dtypes.bfloat16)
    for g in range(NG):
        blk = (nidx[g * NPG:(g + 1) * NPG] - NPG * g) % N  # [512, 16]
        flat = blk.reshape(-1).astype(np.int16)            # node-major
        if compact_chunk0:
            j0 = flat[:NI].astype(np.int64)                # chunk-0 entries
            uniq, inv = np.unique(j0, return_inverse=True)
            xcomp[g, :, :len(uniq), :] = xe4[g][:, uniq, :]
            flat = flat.copy()
            flat[:NI] = inv.astype(np.int16)               # remapped idx
        blocks.append(flat.reshape(-1, 16).T)              # [16, 512]
    idx = np.concatenate(blocks, axis=0)                   # [128, 512]
    if xcomp is not None:
        xcomp = np.ascontiguousarray(xcomp.reshape(C, NI * D))
    return xe, np.ascontiguousarray(idx), xcomp


def _run(x: np.ndarray, neighbor_idx: np.ndarray, **spmd_kwargs):
    x = np.asarray(x, dtype=np.float32)
    neighbor_idx = np.asarray(neighbor_idx)

    if "nc" not in _NC_CACHE:
        _NC_CACHE["nc"] = _build_program()
    nc = _NC_CACHE["nc"]

    in_maps = []
    for core in range(N_CORES):
        lo = core * S
        xes, idxs = [], []
        xcomp = None
        for s in range(S):
            xe, idx, xc = _prep_sample(x[lo + s], neighbor_idx[lo + s],
                                       compact_chunk0=(s == 0))
            xes.append(xe)
            idxs.append(idx)
            if xc is not None:
                xcomp = xc
        in_maps.append({
            "xe": np.stack(xes, axis=0),
            "idx": np.stack(idxs, axis=0),
            "xcomp": xcomp,
        })

    res = run_bass_kernel_spmd(nc, in_maps, core_ids=list(range(N_CORES)),
                               **spmd_kwargs)
    out = np.concatenate([res.results[core]["out"] for core in range(N_CORES)],
                         axis=0)
    return out.astype(np.float32), res


def kernel(x: np.ndarray, neighbor_idx: np.ndarray) -> np.ndarray:
    return _run(x, neighbor_idx)[0]


if __name__ == "__main__":
    rng = np.random.default_rng(0)
    xt = rng.standard_normal((B, C, N)).astype(np.float32)
    it = rng.integers(0, N, size=(B, N, K)).astype(np.int64)
    got = kernel(xt, it)
    ref = np.maximum(
        np.max(xt[np.arange(B)[:, None, None], :, it], axis=2).transpose(0, 2, 1),
        xt,
    )
    xb = xt.astype(ml_dtypes.bfloat16).astype(np.float32)
    refb = np.maximum(
        np.max(xb[np.arange(B)[:, None, None], :, it], axis=2).transpose(0, 2, 1),
        xb,
    )
    print("abs err vs f32 ref:", np.abs(got - ref).max())
    print("abs err vs bf16 ref:", np.abs(got - refb).max())



# revision 26
# speedup vs baseline: 1.0035x; 1.0035x over previous
"""GNN neighbor-max kernel — bf16 ap_gather + pair-max-tree design.

Per core: 2 samples, batch-parallel across the 8 NeuronCores. Per sample:
  bf16 table xe[16g+q, m, j] = x[8q+j, (m + 512g) % N]  (one copy per GPSIMD
  group, rolled by the group's node base so each group's own nodes sit at
  uniform offsets; 64KB/partition, double-buffered across samples).
  Group g owns nodes [g*512, (g+1)*512); its per-chunk index list holds the
  16 neighbors of 32 nodes, pre-shifted by -512g mod N: one ap_gather of 512
  idx -> gt [128, 512*8] bf16.
  DVE reduces k=16 via a contiguous pair-max tree (16->8->4->2->1, 2-byte
  packed innermost so the DVE 2x mode applies), then a final max against the
  table's own x slice (self node, uniform offset thanks to the roll) writes
  transposed into oblk [128, (j, n)].
  Per-sample oblk buffers; scalar engine drains uneven waves (5/8 then 3/8 of
  nodes) to out[C, N] bf16 so the post-last-gather tail is short.
  A tiny warmup ap_gather on zeroed scratch runs before the table wait to
  pre-pay MODIFY_POOL_CONFIG + the ~6us ucode IRAM load.

  Perf notes (HW-measured): the kernel is bound by ap_gather at ~39 cyc/idx
  (~63 GB/s/core of gathered bytes); dma_gather alternatives measure 26-31
  GB/s (SWDGE descriptor-generation bound) and multi-queue SWDGE corrupts
  data, so this architecture is the practical optimum with stock ucode.
"""

import numpy as np
import ml_dtypes

import concourse.bacc as bacc
import concourse.bass as bass
import concourse.mybir as mybir
from concourse.bass_utils import run_bass_kernel_spmd

B, C, N, K = 16, 128, 4096, 16
N_CORES = 8
S = B // N_CORES
D = 8                      # channels per partition block
NG = 8                     # gpsimd groups
NPG = N // NG              # 512 nodes per group
CHUNKS = 8
NPC = NPG // CHUNKS        # 32 nodes per group per chunk
NI = NPC * K               # 512 idx per gather
NCOLS = NPG * K // 16      # 512 idx columns per sample per partition

_NC_CACHE = {}


def _build_program():
    nc = bacc.Bacc(None, target_bir_lowering=False)

    bf16 = mybir.dt.bfloat16
    xe_d = nc.dram_tensor("xe", [S, C, N * D], bf16, kind="ExternalInput")
    idx_d = nc.dram_tensor("idx", [S, C, NCOLS], mybir.dt.int16,
                           kind="ExternalInput")
    # compact dedup'd table for sample-0 chunk-0 (starts the gather stream
    # ~16us before the full tbl0 lands); loaded into gt2 (dead until chunk 2)
    xcomp_d = nc.dram_tensor("xcomp", [C, NI * D], bf16, kind="ExternalInput")
    out_d = nc.dram_tensor("out", [S, C, N], bf16, kind="ExternalOutput")

    with (
        nc.Block() as block,
        nc.semaphore("isem") as isem,   # idx DMAs
        nc.semaphore("csem") as csem,   # compact chunk-0 table DMA
        nc.semaphore("tsem") as tsem,   # table DMAs
        nc.semaphore("gsem") as gsem,   # gather chunks done
        nc.semaphore("bsem") as bsem,   # tree level A done (gt free)
        nc.semaphore("msem") as msem,   # per-half-sample maxes done
        nc.semaphore("osem") as osem,   # out DMAs done
        nc.sbuf_tensor("tbl0", [C, N * D], bf16) as tbl0,      # 64KB/p
        nc.sbuf_tensor("tbl1", [C, N * D], bf16) as tbl1,      # 64KB/p
        nc.sbuf_tensor("gt0", [C, NI * D], bf16) as gt0,       # 16KB/p
        nc.sbuf_tensor("gt1", [C, NI * D], bf16) as gt1,
        nc.sbuf_tensor("gt2", [C, NI * D], bf16) as gt2,
        nc.sbuf_tensor("tA", [C, NPC * 8 * D], bf16) as tA,    # 8KB/p
        nc.sbuf_tensor("tB", [C, NPC * 4 * D], bf16) as tB,
        nc.sbuf_tensor("ob0", [C, D * NPG], bf16) as ob0,      # 8KB/p
        nc.sbuf_tensor("ob1", [C, D * NPG], bf16) as ob1,
        nc.sbuf_tensor("idxt", [C, S * NCOLS], mybir.dt.int16) as idxt,
        nc.sbuf_tensor("wg", [C, 32], mybir.dt.int16) as wg,
        nc.sbuf_tensor("wo", [C, 32], mybir.dt.int16) as wo,
    ):
        tbls = [tbl0, tbl1]
        gts = [gt0, gt1, gt2]
        obs = [ob0, ob1]

        def drain_wave(eng, s, h, r0, r1, groups):
            for gg in groups:
                src = obs[s][gg * 16:(gg + 1) * 16].rearrange(
                    "p (j n) -> p j n", j=D)[:, :, r0:r1]
                dst = bass.AP(
                    out_d,
                    s * C * N + gg * NPG + r0,
                    [[D * N, 16], [N, D], [1, r1 - r0]],
                )
                eng.dma_start(out=dst, in_=src).then_inc(osem, 16)

        waves = [(0, 5 * NPC), (5 * NPC, NPG)]

        @block.sync
        def _(sy: bass.BassEngine):
            for s in range(S):
                sy.dma_start(out=idxt[:, s * NCOLS:(s + 1) * NCOLS],
                             in_=idx_d[s]).then_inc(isem, 16)
            sy.dma_start(out=gts[2][:], in_=xcomp_d[:]).then_inc(csem, 16)
            for s in range(S):
                sy.dma_start(out=tbls[s][:], in_=xe_d[s]).then_inc(tsem, 16)
            # sync engine drains groups 4-7 of each wave in parallel with
            # the scalar engine's groups 0-3 (halves the issue serialization)
            for s in range(S):
                for h, (r0, r1) in enumerate(waves):
                    sy.wait_ge(msem, 2 * s + h + 1)
                    drain_wave(sy, s, h, r0, r1, range(NG // 2, NG))
            sy.wait_ge(osem, 16 * NG * 2 * S)

        @block.gpsimd
        def _(g: bass.BassGpSimd):
            # warmup: pre-pay MODIFY_POOL_CONFIG + ~6us IRAM load while the
            # tables are still streaming in (gathers 16 zero-idx from zeros)
            g.memset(wg[:], 0)
            g.ap_gather(
                out_ap=wo[:].rearrange("p (n j) -> p n j", j=2),
                in_ap=wg[:].rearrange("p (n j) -> p n j", j=2),
                idxs_ap=wg[:, 0:1],
                channels=C, num_elems=16, d=2, num_idxs=16,
            )
            g.wait_ge(isem, 16 * S)
            # chunk (0,0) gathers from the compact table in gt2 — starts as
            # soon as the 2MB compact DMA lands, well before tbl0
            g.wait_ge(csem, 16)
            g.ap_gather(
                out_ap=gts[0][:],
                in_ap=gts[2][:],
                idxs_ap=idxt[:, 0:NI // 16],
                channels=C, num_elems=NI, d=D, num_idxs=NI,
            ).then_inc(gsem, 1)
            for s in range(S):
                g.wait_ge(tsem, 16 * (s + 1))
                for c in range(CHUNKS):
                    ci = s * CHUNKS + c
                    if ci == 0:
                        continue
                    if ci >= 3:
                        g.wait_ge(bsem, ci - 2)
                    col0 = s * NCOLS + c * (NI // 16)
                    g.ap_gather(
                        out_ap=gts[ci % 3][:],
                        in_ap=tbls[s][:],
                        idxs_ap=idxt[:, col0:col0 + NI // 16],
                        channels=C, num_elems=N, d=D, num_idxs=NI,
                    ).then_inc(gsem, 1)

        @block.vector
        def _(v: bass.BassVectorEngine):
            def tree(s, c, buf, n0, npc, gtarget, inc_msem):
                """Pair-max tree over nodes [n0, n0+npc) of chunk (s, c)."""
                v.wait_ge(gsem, gtarget)
                gv = gts[buf][:, n0 * K * D:(n0 + npc) * K * D].rearrange(
                    "p (n k j) -> p n k j", k=K, j=D)
                av = tA[:, :npc * 8 * D].rearrange(
                    "p (n t j) -> p n t j", t=8, j=D)
                bv = tB[:, :npc * 4 * D].rearrange(
                    "p (n t j) -> p n t j", t=4, j=D)
                # C/D reuse tA: op B consumed all of tA before C writes it
                cv = tA[:, :npc * 2 * D].rearrange(
                    "p (n t j) -> p n t j", t=2, j=D)
                dv = tA[:, npc * 2 * D:npc * 3 * D].rearrange(
                    "p (n j) -> p n j", j=D)
                # half-pairing (k vs k+8, then t vs t+4, ...) keeps operands
                # as long contiguous runs (64/32/16 elems) for DVE streaming;
                # max is commutative so the result is bit-identical
                v.tensor_max(out=av, in0=gv[:, :, 0:8, :],
                             in1=gv[:, :, 8:16, :]).then_inc(bsem, 1)
                v.tensor_max(out=bv, in0=av[:, :, 0:4, :],
                             in1=av[:, :, 4:8, :])
                v.tensor_max(out=cv, in0=bv[:, :, 0:2, :],
                             in1=bv[:, :, 2:4, :])
                v.tensor_max(out=dv, in0=cv[:, :, 0, :],
                             in1=cv[:, :, 1, :])
                lo = c * NPC + n0
                sv = tbls[s][:].rearrange("p (n j) -> p n j", j=D)[
                    :, lo:lo + npc, :]
                ov = obs[s][:].rearrange("p (j n) -> p j n", j=D)
                ov = ov.transpose([0, 2, 1])[:, lo:lo + npc, :]
                e = v.tensor_max(out=ov, in0=dv, in1=sv)
                if inc_msem:
                    e.then_inc(msem, 1)

            for s in range(S):
                for c in range(CHUNKS):
                    ci = s * CHUNKS + c
                    if ci == 0:
                        # chunk 0 gathered from the compact table; its
                        # self-max still reads tbl0 — ensure it landed
                        v.wait_ge(tsem, 16)
                    tree(s, c, ci % 3, 0, NPC, ci + 1,
                         c == 4 or c == CHUNKS - 1)

        @block.scalar
        def _(sc: bass.BassEngine):
            # uneven waves: first drain covers chunks 0-4 (5*NPC nodes) so the
            # post-last-gather tail only moves the remaining 3*NPC nodes
            for s in range(S):
                for h, (r0, r1) in enumerate(waves):
                    sc.wait_ge(msem, 2 * s + h + 1)
                    drain_wave(sc, s, h, r0, r1, range(NG // 2))
            sc.wait_ge(osem, 16 * NG * 2 * S)

    nc.compile()
    return nc


def _prep_sample(x_s: np.ndarray, nidx_s: np.ndarray, compact_chunk0=False):
    """x_s [C, N] f32, nidx_s [N, K] int -> (xe [C, N*D] bf16, idx [C, NCOLS]
    i16, xcomp [C, NI*D] bf16 | None).

    With compact_chunk0, chunk 0's idx are remapped into a dedup'd compact
    column table (xcomp) so the first gather can run before the full table
    DMA completes."""
    xq = x_s.reshape(16, D, N).transpose(0, 2, 1)          # [q, n, j]
    xe4 = np.empty((NG, 16, N, D), dtype=np.float32)
    for g in range(NG):
        xe4[g] = np.roll(xq, -NPG * g, axis=1)             # group-rolled copy
    xe4 = xe4.astype(ml_dtypes.bfloat16)
    xe = np.ascontiguousarray(xe4.reshape(C, N * D))
    nidx = np.asarray(nidx_s, dtype=np.int64)              # [N, K]
    blocks = []
    xcomp = None
    if compact_chunk0:
        xcomp = np.zeros((NG, 16, NI, D), dtype=ml_dtypes.bfloat16)
    for g in range(NG):
        blk = (nidx[g * NPG:(g + 1) * NPG] - NPG * g) % N  # [512, 16]
        flat = blk.reshape(-1).astype(np.int16)            # node-major
        if compact_chunk0:
            j0 = flat[:NI].astype(np.int64)                # chunk-0 entries
            uniq, inv = np.unique(j0, return_inverse=True)
            xcomp[g, :, :len(uniq), :] = xe4[g][:, uniq, :]
            flat = flat.copy()
            flat[:NI] = inv.astype(np.int16)               # remapped idx
        blocks.append(flat.reshape(-1, 16).T)              # [16, 512]
    idx = np.concatenate(blocks, axis=0)                   # [128, 512]
    if xcomp is not None:
        xcomp = np.ascontiguousarray(xcomp.reshape(C, NI * D))
    return xe, np.ascontiguousarray(idx), xcomp


def _run(x: np.ndarray, neighbor_idx: np.ndarray, **spmd_kwargs):
    x = np.asarray(x, dtype=np.float32)
    neighbor_idx = np.asarray(neighbor_idx)

    if "nc" not in _NC_CACHE:
        _NC_CACHE["nc"] = _build_program()
    nc = _NC_CACHE["nc"]

    in_maps = []
    for core in range(N_CORES):
        lo = core * S
        xes, idxs = [], []
        xcomp = None
        for s in range(S):
            xe, idx, xc = _prep_sample(x[lo + s], neighbor_idx[lo + s],
                                       compact_chunk0=(s == 0))
            xes.append(xe)
            idxs.append(idx)
            if xc is not None:
                xcomp = xc
        in_maps.append({
            "xe": np.stack(xes, axis=0),
            "idx": np.stack(idxs, axis=0),
            "xcomp": xcomp,
        })

    res = run_bass_kernel_spmd(nc, in_maps, core_ids=list(range(N_CORES)),
                               **spmd_kwargs)
    out = np.concatenate([res.results[core]["out"] for core in range(N_CORES)],
                         axis=0)
    return out.astype(np.float32), res


def kernel(x: np.ndarray, neighbor_idx: np.ndarray) -> np.ndarray:
    return _run(x, neighbor_idx)[0]


if __name__ == "__main__":
    rng = np.random.default_rng(0)
    xt = rng.standard_normal((B, C, N)).astype(np.float32)
    it = rng.integers(0, N, size=(B, N, K)).astype(np.int64)
    got = kernel(xt, it)
    ref = np.maximum(
        np.max(xt[np.arange(B)[:, None, None], :, it], axis=2).transpose(0, 2, 1),
        xt,
    )
    xb = xt.astype(ml_dtypes.bfloat16).astype(np.float32)
    refb = np.maximum(
        np.max(xb[np.arange(B)[:, None, None], :, it], axis=2).transpose(0, 2, 1),
        xb,
    )
    print("abs err vs f32 ref:", np.abs(got - ref).max())
    print("abs err vs bf16 ref:", np.abs(got - refb).max())

